# revision 5
# baseline (speedup 1.0000x reference)
"""LocalContrastNormalization Trainium2 kernel.

Sharding: pure data parallel, batch 32 -> 4 images per core across 8 cores.
Per image (512x512x3): channel-sum -> 9x9 'SAME' conv via 9 fp32r band
matmuls per 128-row tile (+1 merged boundary matmul fed by a DMA-replicated
4-row scratch), v = x - local_mean, sigma^2 likewise from sum_c v^2,
1/sigma = exp(-0.5 ln sigma^2) on ScalarE, mean(sigma) via fused
tensor_tensor_reduce row-sums, final out = v * min(1/sigma, 1/mean_sigma).
"""
import numpy as np
import bass_rust
import concourse.bacc as bacc
import concourse.mybir as mybir
import concourse.tile as tile
from concourse import bass_utils

F32 = mybir.dt.float32
F32R = mybir.dt.float32r
AX = mybir.AxisListType
OP = mybir.AluOpType
ACT = mybir.ActivationFunctionType

B, H, W, C = 32, 512, 512, 3
NCORES = 8
IPC = B // NCORES          # images per core
NT = H // 128              # 128-row tiles per image
WC = W * C

_cache = {}


def _build():
    nc = bacc.Bacc("TRN2", debug=False)
    x_d = nc.dram_tensor("x", [IPC, H, WC], F32, kind="ExternalInput")
    bm_d = nc.dram_tensor("bm", [9, 128, 128], F32R, kind="ExternalInput")
    btb_d = nc.dram_tensor("btb", [72, 128], F32R, kind="ExternalInput")
    bb36_d = nc.dram_tensor("bb36", [36, 128], F32R, kind="ExternalInput")
    z_d = nc.dram_tensor("z8", [128, 8], F32R, kind="ExternalInput")
    o_d = nc.dram_tensor("out", [IPC, H, WC], F32, kind="ExternalOutput")

    with tile.TileContext(nc) as tc:
        with tc.tile_pool(name="const", bufs=1) as cp, \
             tc.tile_pool(name="xp", bufs=6) as xp, \
             tc.tile_pool(name="mp", bufs=6) as mp, \
             tc.tile_pool(name="v2p", bufs=2) as v2p, \
             tc.tile_pool(name="wp", bufs=6) as wp, \
             tc.tile_pool(name="tp", bufs=5) as tp, \
             tc.tile_pool(name="sp", bufs=2) as sp, \
             tc.tile_pool(name="bp", bufs=3) as bp, \
             tc.tile_pool(name="scp", bufs=3) as scp, \
             tc.tile_pool(name="stp", bufs=2) as stp, \
             tc.tile_pool(name="ps1", bufs=2, space="PSUM") as ps1, \
             tc.tile_pool(name="ps2", bufs=2, space="PSUM") as ps2:

            bm = cp.tile([128, 9 * 128], F32R)
            nc.sync.dma_start(bm[:].rearrange("p (d m) -> p d m", d=9),
                              bm_d.ap()[:].rearrange("d p m -> p d m"))
            btb = cp.tile([72, 128], F32R)
            nc.sync.dma_start(btb[:], btb_d.ap()[:])
            bb36 = cp.tile([36, 128], F32R)
            nc.sync.dma_start(bb36[:], bb36_d.ap()[:])
            zc = cp.tile([128, 8], F32R)
            nc.sync.dma_start(zc[:], z_d.ap()[:])

            def conv(src_tiles, t, pspool):
                """9 main band matmuls + merged boundary matmul -> psum tile."""
                psum = pspool.tile([128, W], F32)
                for dx in range(9):
                    nc.tensor.matmul(psum[:], bm[:, 128 * dx:128 * (dx + 1)],
                                     src_tiles[t][:, dx:dx + W],
                                     start=(dx == 0), stop=False)
                # boundary scratch: rows outside this tile, replicated 9x with
                # the dx shift applied by the broadcast DMA
                def repl(rows_ap, dst_ap):
                    bounce = bp.tile([1, 2080], F32R)
                    nc.sync.dma_start(bounce[:], rows_ap)
                    src = bass_rust.AP(tensor=bounce[:].tensor,
                                       offset=bounce[:].offset,
                                       ap=[[1, 9], [520, 4], [1, 512]])
                    nc.sync.dma_start(dst_ap, src)
                scr = scp.tile([72, W], F32R)
                if t == 0:
                    repl(src_tiles[1][0:4, :], scr[0:36, :])
                    nc.tensor.matmul(psum[:], bb36[:], scr[0:36, :],
                                     start=False, stop=True)
                elif t == NT - 1:
                    repl(src_tiles[t - 1][124:128, :], scr[0:36, :])
                    nc.tensor.matmul(psum[:], btb[0:36, :], scr[0:36, :],
                                     start=False, stop=True)
                else:
                    repl(src_tiles[t - 1][124:128, :], scr[0:36, :])
                    repl(src_tiles[t + 1][0:4, :], scr[36:72, :])
                    nc.tensor.matmul(psum[:], btb[:], scr[:],
                                     start=False, stop=True)
                return psum

            for i in range(IPC):
                xts, msums = [], []
                for t in range(NT):
                    xt = xp.tile([128, WC], F32)
                    nc.sync.dma_start(xt[:], x_d.ap()[i, 128 * t:128 * (t + 1), :])
                    xts.append(xt)
                    ms = mp.tile([128, 520], F32R)
                    nc.sync.dma_start(ms[:, 0:4], zc[:, 0:4])
                    nc.sync.dma_start(ms[:, 516:520], zc[:, 4:8])
                    with nc.allow_low_precision(reason="f32r conv input"):
                        nc.vector.tensor_reduce(
                            ms[:, 4:516], xt[:].rearrange("p (j c) -> p j c", c=3),
                            axis=AX.X, op=OP.add)
                    msums.append(ms)

                vsums = []
                for t in range(NT):
                    p1 = conv(msums, t, ps1)
                    xv = xts[t][:].rearrange("p (j c) -> p j c", c=3)
                    lmb = p1[:].unsqueeze(-1).broadcast_to((128, W, 3))
                    nc.vector.tensor_tensor(xv, xv, lmb, op=OP.subtract)  # x -> v
                    v2 = v2p.tile([128, WC], F32)
                    nc.scalar.square(v2[:], xts[t][:])
                    vs = wp.tile([128, 520], F32R)
                    nc.sync.dma_start(vs[:, 0:4], zc[:, 0:4])
                    nc.sync.dma_start(vs[:, 516:520], zc[:, 4:8])
                    with nc.allow_low_precision(reason="f32r conv input"):
                        nc.vector.tensor_reduce(
                            vs[:, 4:516], v2[:].rearrange("p (j c) -> p j c", c=3),
                            axis=AX.X, op=OP.add)
                    vsums.append(vs)

                stats = stp.tile([128, NT], F32)
                invs = []
                for t in range(NT):
                    p2 = conv(vsums, t, ps2)
                    t2 = tp.tile([128, W], F32)
                    nc.scalar.activation(t2[:], p2[:], ACT.Ln)
                    nc.scalar.activation(t2[:], t2[:], ACT.Exp, scale=-0.5)
                    scrap = sp.tile([128, W], F32)
                    nc.vector.tensor_tensor(scrap[:], p2[:], t2[:], op=OP.mult)
                    nc.vector.tensor_reduce(stats[:, t:t + 1], scrap[:],
                                            axis=AX.X, op=OP.add)
                    invs.append(t2)

                # mean_sigma: flatten stats across partitions via DMA, reduce
                flat = stp.tile([1, 128 * NT], F32)
                nc.sync.dma_start(flat[:], stats[:])
                tot = stp.tile([1, 1], F32)
                nc.vector.tensor_reduce(tot[:], flat[:], axis=AX.X, op=OP.add)
                nc.vector.reciprocal(tot[:], tot[:])
                nc.vector.tensor_scalar_mul(tot[:], tot[:], float(H * W))
                ims = stp.tile([128, 1], F32)
                nc.sync.dma_start(ims[0:1, :], tot[:])
                k = 1
                while k < 128:
                    nc.sync.dma_start(ims[k:2 * k, :], ims[0:k, :])
                    k *= 2

                for t in range(NT):
                    xv = xts[t][:].rearrange("p (j c) -> p j c", c=3)
                    rb = invs[t][:].unsqueeze(-1).broadcast_to((128, W, 3))
                    nc.vector.scalar_tensor_tensor(
                        xv, rb, ims[:, 0:1], xv, op0=OP.min, op1=OP.mult)
                    nc.sync.dma_start(o_d.ap()[i, 128 * t:128 * (t + 1), :], xts[t][:])
    nc.compile()
    return nc


def _bands(k9):
    K2 = (k9 / 3.0).astype(np.float32)
    bm = np.zeros((9, 128, 128), np.float32)
    for dx in range(9):
        for k in range(128):
            for p in range(max(0, k - 4), min(128, k + 5)):
                bm[dx, k, p] = K2[k - p + 4, dx]
    top = np.zeros((36, 128), np.float32)
    bot = np.zeros((36, 128), np.float32)
    for dx in range(9):
        for r in range(4):
            for p in range(0, r + 1):
                top[dx * 4 + r, p] = K2[r - p, dx]
            for p in range(124 + r, 128):
                bot[dx * 4 + r, p] = K2[132 + r - p, dx]
    return bm, np.concatenate([top, bot], 0), bot


def kernel(x, kernel):
    x = np.ascontiguousarray(np.asarray(x, np.float32))
    k9 = np.asarray(kernel, np.float32)[:, :, 0, 0]
    if "nc" not in _cache:
        _cache["nc"] = _build()
    nc = _cache["nc"]
    bm, btb, bb36 = _bands(k9)
    xs = x.reshape(NCORES, IPC, H, WC)
    z8 = np.zeros((128, 8), np.float32)
    in_maps = [{"x": xs[c], "bm": bm, "btb": btb, "bb36": bb36, "z8": z8}
               for c in range(NCORES)]
    res = bass_utils.run_bass_kernel_spmd(nc, in_maps, core_ids=list(range(NCORES)))
    out = np.stack([res.results[c]["out"] for c in range(NCORES)])
    return out.reshape(B, H, W, C)
